# revision 11
# baseline (speedup 1.0000x reference)
"""Two-layer GCN on 8 Trainium2 NeuronCores (Bass/Tile).

Math (reference, per layer):
    deg  = segment_sum(ones, dst)                 # target-side degrees
    dinv = where(deg>0, rsqrt(deg), 0)
    out[d] = dinv[d] * sum_{e: dst[e]=d} dinv[src[e]] * x[src[e]]  @ W  + b
(the x@W GEMM commutes with the segment sum, so we aggregate raw features
and apply W once per 128-node output block).

Distribution: dst nodes (and their incident edges) are sharded across the 8
cores; x (fp16) is replicated in every core's HBM.  Per 128-edge chunk the
kernel gathers the source rows with dma_gather, builds a dinv-weighted
one-hot selection matrix [128e x 128dst] on DVE, and scatter-adds via a
TensorE matmul accumulating in PSUM: psum[f, d] += gathered.T @ sel.  A
second matmul applies the layer weight; the layer-1 activations are
exchanged with an AllGather so layer 2 can gather any source row.

dma_gather indices are int16, so sources are split into lo (< 32768) and
hi (>= 32768) edge lists; the hi gather reads from a view of the feature
table offset by 32768 rows.
"""

import os
import sys
import time

sys.path.insert(0, "/opt/trn_rl_repo")

import numpy as np

import concourse.bass as bass
import concourse.bacc as bacc
import concourse.tile as tile
from concourse import mybir
from concourse.bass_utils import run_bass_kernel_spmd

P = 128
N_NODES = 50000
N_EDGES = 800000
IN_DIM = 128
HID_DIM = 128
OUT_DIM = 64
NCORES = 8
SHARD = N_NODES // NCORES          # 6250
NBLK = (SHARD + P - 1) // P        # 49 dst blocks per core (48 full + 106)
SPLIT = 32768                      # int16 index limit
SB_BLOCKS = 7                      # dst blocks per superblock (gather batch)
LAST_ROWS = SHARD - (NBLK - 1) * P # rows in the final dst block

# Filled by kernel() on the last run (for test.py introspection).
LAST_RESULTS = None


# --------------------------------------------------------------------------
# Host-side preprocessing
# --------------------------------------------------------------------------

def _wrap_idx(idx_chunks):
    """int16 indices for one dma_gather call: [16, n/16] wrap replicated to
    128 partitions.  idx_chunks: int array [n_chunks, 128]."""
    flat = idx_chunks.reshape(-1)
    n = flat.shape[0]
    arr = flat.reshape(n // 16, 16).T.astype(np.int16)   # [16, n/16]
    return np.tile(arr, (8, 1))                          # [128, n/16]


def preprocess(x, edge_index, W1, b1, W2, b2):
    x = np.asarray(x, dtype=np.float32)
    edge_index = np.asarray(edge_index).astype(np.int64)
    src_g = edge_index[0].astype(np.int32)
    dst_g = edge_index[1].astype(np.int32)

    deg = np.bincount(dst_g, minlength=N_NODES).astype(np.float32)
    dinv = np.where(deg > 0, 1.0 / np.sqrt(np.maximum(deg, 1.0)), 0.0).astype(
        np.float32
    )

    # per (core, block, lo/hi) edge lists
    owner = dst_g // SHARD
    blk_loc = (dst_g % SHARD) // P
    rel = (dst_g % SHARD) % P
    is_lo = src_g < SPLIT

    # bucket sort edges by (owner, block, hi/lo)
    key = ((owner * NBLK + blk_loc) * 2 + (~is_lo).astype(np.int32)).astype(np.int64)
    order = np.argsort(key, kind="stable")
    key_s = key[order]
    src_s = src_g[order]
    rel_s = rel[order]
    bounds = np.searchsorted(key_s, np.arange(NCORES * NBLK * 2 + 1))

    def bucket(c, b, hi):
        k = (c * NBLK + b) * 2 + hi
        lo_i, hi_i = bounds[k], bounds[k + 1]
        return src_s[lo_i:hi_i], rel_s[lo_i:hi_i]

    nchunks = np.zeros((NCORES, NBLK, 2), np.int64)
    for c in range(NCORES):
        for b in range(NBLK):
            for h in (0, 1):
                n = bounds[(c * NBLK + b) * 2 + h + 1] - bounds[(c * NBLK + b) * 2 + h]
                nchunks[c, b, h] = -(-n // P)
    cap = nchunks.max(axis=0)                       # [NBLK, 2] shared structure
    cap[:, 0] = np.maximum(cap[:, 0], 1)            # >=1 chunk per block

    # superblock structure
    sbs = [list(range(s, min(s + SB_BLOCKS, NBLK))) for s in range(0, NBLK, SB_BLOCKS)]
    meta = {
        "cap": cap,
        "sbs": sbs,
        "has_b1": bool(np.any(np.asarray(b1))),
        "has_b2": bool(np.any(np.asarray(b2))),
    }

    # per-core arrays.  x is pre-scaled by dinv[src] so the selection matrix
    # is a plain one-hot (single DVE op); padded lanes get dstrel=-1, which
    # matches no iota column and therefore contributes nothing.
    total_lo = int(cap[:, 0].sum())
    total_hi = int(cap[:, 1].sum())
    in_maps = []
    x16 = (dinv[:, None] * x).astype(np.float16)
    for c in range(NCORES):
        idx_lo = np.zeros((total_lo, P), np.int32)
        idx_hi = np.zeros((total_hi, P), np.int32)
        m_dst = np.zeros((P, total_lo + total_hi), np.float32)
        off = {0: 0, 1: 0}
        for b in range(NBLK):
            for h in (0, 1):
                s_arr, r_arr = bucket(c, b, h)
                n = s_arr.shape[0]
                ncap = int(cap[b, h])
                idxs = np.zeros(ncap * P, np.int32)
                idxs[:n] = s_arr - (SPLIT if h else 0)
                d_arr = np.full(ncap * P, -1.0, np.float32)
                d_arr[:n] = r_arr
                o = off[h]
                tgt = idx_lo if h == 0 else idx_hi
                tgt[o : o + ncap] = idxs.reshape(ncap, P)
                col0 = o if h == 0 else total_lo + o
                m_dst[:, col0 : col0 + ncap] = d_arr.reshape(ncap, P).T
                off[h] += ncap

        # wrap gather indices per superblock call
        lo_cols, hi_cols = [], []
        lo_o = hi_o = 0
        for sb in sbs:
            n_lo = int(cap[sb, 0].sum())
            n_hi = int(cap[sb, 1].sum())
            lo_cols.append(_wrap_idx(idx_lo[lo_o : lo_o + n_lo]))
            lo_o += n_lo
            if n_hi:
                hi_cols.append(_wrap_idx(idx_hi[hi_o : hi_o + n_hi]))
                hi_o += n_hi
        idx_lo_w = np.concatenate(lo_cols, axis=1)
        idx_hi_w = (
            np.concatenate(hi_cols, axis=1)
            if hi_cols
            else np.zeros((P, 8), np.int16)
        )

        tmp = np.zeros(NBLK * P, np.float32)
        tmp[:SHARD] = dinv[c * SHARD : (c + 1) * SHARD]
        dinvd = tmp.reshape(NBLK, P).T.copy()   # [p, b] = dinv[c*SHARD + b*P + p]

        im = {
            "x16": x16,
            "idx_lo": idx_lo_w,
            "idx_hi": idx_hi_w,
            "m_dst": m_dst,
            "m_dstn": -m_dst,
            "dinvd": dinvd,
            "dinvd2": dinvd * dinvd,
            "w1": np.asarray(W1, np.float32).astype(np.float16),
            "w2": np.asarray(W2, np.float32).astype(np.float16),
        }
        if meta["has_b1"]:
            im["b1rep"] = np.tile(np.asarray(b1, np.float32)[None, :], (P, 1))
        if meta["has_b2"]:
            im["b2rep"] = np.tile(np.asarray(b2, np.float32)[None, :], (P, 1))
        in_maps.append(im)
    return meta, in_maps


# --------------------------------------------------------------------------
# Bass kernel
# --------------------------------------------------------------------------

def build(meta):
    cap = meta["cap"]
    sbs = meta["sbs"]
    total_lo = int(cap[:, 0].sum())
    total_hi = int(cap[:, 1].sum())

    nc = bacc.Bacc(
        "TRN2",
        target_bir_lowering=False,
        debug=False,
        enable_asserts=True,
        num_devices=NCORES,
        num_swdge_queues=4,
    )
    x16 = nc.dram_tensor("x16", [N_NODES, IN_DIM], mybir.dt.float16, kind="ExternalInput")
    idx_lo_d = nc.dram_tensor("idx_lo", [P, total_lo * 8], mybir.dt.int16, kind="ExternalInput")
    idx_hi_d = nc.dram_tensor(
        "idx_hi", [P, max(total_hi, 1) * 8], mybir.dt.int16, kind="ExternalInput"
    )
    m_dst_d = nc.dram_tensor("m_dst", [P, total_lo + total_hi], mybir.dt.float32, kind="ExternalInput")
    m_dstn_d = nc.dram_tensor("m_dstn", [P, total_lo + total_hi], mybir.dt.float32, kind="ExternalInput")
    dinvd_d = nc.dram_tensor("dinvd", [P, NBLK], mybir.dt.float32, kind="ExternalInput")
    dinvd2_d = nc.dram_tensor("dinvd2", [P, NBLK], mybir.dt.float32, kind="ExternalInput")
    w1_d = nc.dram_tensor("w1", [IN_DIM, HID_DIM], mybir.dt.float16, kind="ExternalInput")
    w2_d = nc.dram_tensor("w2", [HID_DIM, OUT_DIM], mybir.dt.float16, kind="ExternalInput")
    b1_d = (
        nc.dram_tensor("b1rep", [P, HID_DIM], mybir.dt.float32, kind="ExternalInput")
        if meta["has_b1"]
        else None
    )
    b2_d = (
        nc.dram_tensor("b2rep", [P, OUT_DIM], mybir.dt.float32, kind="ExternalInput")
        if meta["has_b2"]
        else None
    )
    out_d = nc.dram_tensor("out", [SHARD, OUT_DIM], mybir.dt.float32, kind="ExternalOutput")

    with tile.TileContext(nc) as tc:
        with (
            tc.tile_pool(name="const", bufs=1) as const,
            tc.tile_pool(name="gpool", bufs=2) as gpool,
            tc.tile_pool(name="selp", bufs=16) as selp,
            tc.tile_pool(name="sbuf", bufs=3) as sbp,
            tc.tile_pool(name="psA", bufs=4, space="PSUM") as psA,
            tc.tile_pool(name="psB", bufs=2, space="PSUM") as psB,
            tc.tile_pool(name="dram", bufs=1, space="DRAM") as dram,
        ):
            # ---- one-time loads
            idx_lo_sb = const.tile([P, total_lo * 8], mybir.dt.int16)
            nc.sync.dma_start(out=idx_lo_sb[:], in_=idx_lo_d[:])
            idx_hi_sb = const.tile([P, max(total_hi, 1) * 8], mybir.dt.int16)
            nc.sync.dma_start(out=idx_hi_sb[:], in_=idx_hi_d[:])
            m_dst_sb = const.tile([P, total_lo + total_hi], mybir.dt.float32)
            nc.sync.dma_start(out=m_dst_sb[:], in_=m_dst_d[:])
            m_dstn_sb = const.tile([P, total_lo + total_hi], mybir.dt.float32)
            nc.sync.dma_start(out=m_dstn_sb[:], in_=m_dstn_d[:])
            dinvd_sb = const.tile([P, NBLK], mybir.dt.float32)
            nc.sync.dma_start(out=dinvd_sb[:], in_=dinvd_d[:])
            dinvd2_sb = const.tile([P, NBLK], mybir.dt.float32)
            nc.sync.dma_start(out=dinvd2_sb[:], in_=dinvd2_d[:])
            w1_sb = const.tile([IN_DIM, HID_DIM], mybir.dt.float16)
            nc.sync.dma_start(out=w1_sb[:], in_=w1_d[:])
            w2_sb = const.tile([HID_DIM, OUT_DIM], mybir.dt.float16)
            nc.sync.dma_start(out=w2_sb[:], in_=w2_d[:])
            b1_sb = b2_sb = None
            if b1_d is not None:
                b1_sb = const.tile([P, HID_DIM], mybir.dt.float32)
                nc.sync.dma_start(out=b1_sb[:], in_=b1_d[:])
            if b2_d is not None:
                b2_sb = const.tile([P, OUT_DIM], mybir.dt.float32)
                nc.sync.dma_start(out=b2_sb[:], in_=b2_d[:])

            iota32 = const.tile([P, P], mybir.dt.int32)
            nc.gpsimd.iota(iota32[:], pattern=[[1, P]], base=0, channel_multiplier=0)
            iota16 = const.tile([P, P], mybir.dt.float16)
            nc.vector.tensor_copy(out=iota16[:], in_=iota32[:])
            iota32f = const.tile([P, P], mybir.dt.float32)
            nc.vector.tensor_copy(out=iota32f[:], in_=iota32[:])

            h16sh = dram.tile([SHARD, HID_DIM], mybir.dt.float16)
            h16full = dram.tile([N_NODES, HID_DIM], mybir.dt.float16, addr_space="Shared")

            # SWDGE descriptor rings can't hold a whole-superblock gather in
            # one instruction (ring carveout is O(512) descs/engine; the
            # ucode's await_space never succeeds past that) — split calls.
            MAXCH = int(os.environ.get("GCN_GATHER_CHUNKS", "16"))
            qrot = [0]

            def gather_split(dst_tile, src_ap, idx_sb, ch_off, n_ch, fin):
                for k0 in range(0, n_ch, MAXCH):
                    kn = min(MAXCH, n_ch - k0)
                    nc.gpsimd.dma_gather(
                        out_ap=dst_tile[:, k0 : k0 + kn, :],
                        in_ap=src_ap,
                        idxs_ap=idx_sb[:, (ch_off + k0) * 8 : (ch_off + k0 + kn) * 8],
                        num_idxs=kn * P,
                        num_idxs_reg=kn * P,
                        elem_size=fin,
                        single_packet=False,
                        queue_num=qrot[0] % 4,
                    )
                    qrot[0] += 1

            def layer(src_dram, fin, w_sb, fout, bias_sb, relu, sink):
                lo_off = 0          # lo chunk offset (also m_dst/m_w column)
                hi_off = 0
                for sb in sbs:
                    n_lo = int(cap[sb, 0].sum())
                    n_hi = int(cap[sb, 1].sum())
                    glo = gpool.tile([P, n_lo, fin], mybir.dt.float16, tag="glo")
                    gather_split(glo, src_dram[:], idx_lo_sb, lo_off, n_lo, fin)
                    ghi = None
                    if n_hi:
                        ghi = gpool.tile([P, n_hi, fin], mybir.dt.float16, tag="ghi")
                        gather_split(ghi, src_dram[SPLIT:, :], idx_hi_sb, hi_off, n_hi, fin)
                    lo_c = 0
                    hi_c = 0
                    for b in sb:
                        # chunk list for this block: (tile, col in tile, meta col)
                        chunks = []
                        for k in range(int(cap[b, 0])):
                            chunks.append((glo, lo_c + k, lo_off + lo_c + k))
                        for k in range(int(cap[b, 1])):
                            chunks.append(
                                (ghi, hi_c + k, total_lo + hi_off + hi_c + k)
                            )
                        lo_c += int(cap[b, 0])
                        hi_c += int(cap[b, 1])

                        ps_s = psA.tile([P, P], mybir.dt.float32, tag="psA")
                        for j, (gt, gc, mc) in enumerate(chunks):
                            sel = selp.tile([P, P], mybir.dt.float16, tag="sel")
                            if j % 3 == 2:
                                # ACT path: sel = relu(1 - (iota - d)^2), exact
                                # one-hot for integer iota/d; pads (d=-1) -> 0.
                                u = selp.tile([P, P], mybir.dt.float32, tag="selu")
                                nc.scalar.activation(
                                    out=u[:], in_=iota32f[:],
                                    func=mybir.ActivationFunctionType.Square,
                                    bias=m_dstn_sb[:, mc : mc + 1],
                                )
                                nc.scalar.activation(
                                    out=sel[:], in_=u[:],
                                    func=mybir.ActivationFunctionType.Relu,
                                    scale=-1.0, bias=1.0,
                                )
                            else:
                                nc.vector.tensor_scalar(
                                    out=sel[:],
                                    in0=iota16[:],
                                    scalar1=m_dst_sb[:, mc : mc + 1],
                                    scalar2=None,
                                    op0=mybir.AluOpType.is_equal,
                                )
                            nc.tensor.matmul(
                                out=ps_s[:],
                                lhsT=gt[:, gc, :],
                                rhs=sel[:],
                                start=(j == 0),
                                stop=(j == len(chunks) - 1),
                            )
                        sT = sbp.tile([P, P], mybir.dt.float16, tag="sT")
                        nc.vector.tensor_copy(out=sT[:], in_=ps_s[:])
                        ps_h = psB.tile([P, fout], mybir.dt.float32, tag="psB")
                        nc.tensor.matmul(
                            out=ps_h[:], lhsT=sT[:], rhs=w_sb[:], start=True, stop=True
                        )
                        sink(b, ps_h, bias_sb, relu)
                    lo_off += n_lo
                    hi_off += n_hi

            def store(dst_dram, dt, fout, extra_dinv):
                # layer 1 stores h16' = dinv * relu(dinv*z + b1) (the leading
                # dinv is the src-side prescale for layer 2's gather); with
                # b1 == 0 this folds to relu(dinv^2 * z) in one ACT op.
                def sink(b, ps_h, bias_sb, relu):
                    rows = P if b < NBLK - 1 else LAST_ROWS
                    o_t = sbp.tile([P, fout], dt, tag=f"o{dt}")
                    if bias_sb is None:
                        sc = dinvd2_sb if extra_dinv else dinvd_sb
                        nc.scalar.activation(
                            out=o_t[:],
                            in_=ps_h[:],
                            func=(
                                mybir.ActivationFunctionType.Relu
                                if relu
                                else mybir.ActivationFunctionType.Copy
                            ),
                            scale=sc[:, b : b + 1],
                        )
                    else:
                        t1 = sbp.tile([P, fout], mybir.dt.float32, tag="t1")
                        nc.vector.tensor_scalar(
                            out=t1[:],
                            in0=ps_h[:],
                            scalar1=dinvd_sb[:, b : b + 1],
                            scalar2=None,
                            op0=mybir.AluOpType.mult,
                        )
                        nc.vector.tensor_tensor(
                            out=t1[:], in0=t1[:], in1=bias_sb[:], op=mybir.AluOpType.add
                        )
                        if relu:
                            nc.scalar.activation(
                                out=o_t[:],
                                in_=t1[:],
                                func=mybir.ActivationFunctionType.Relu,
                                scale=(
                                    dinvd_sb[:, b : b + 1] if extra_dinv else 1.0
                                ),
                            )
                        elif extra_dinv:
                            nc.vector.tensor_scalar(
                                out=o_t[:],
                                in0=t1[:],
                                scalar1=dinvd_sb[:, b : b + 1],
                                scalar2=None,
                                op0=mybir.AluOpType.mult,
                            )
                        else:
                            nc.vector.tensor_copy(out=o_t[:], in_=t1[:])
                    nc.sync.dma_start(
                        out=dst_dram[b * P : b * P + rows, :], in_=o_t[:rows, :]
                    )

                return sink

            # ---- layer 1: x16 -> h16sh
            layer(x16, IN_DIM, w1_sb, HID_DIM, b1_sb, True, store(h16sh, mybir.dt.float16, HID_DIM, True))

            # ---- exchange
            nc.gpsimd.collective_compute(
                "AllGather",
                mybir.AluOpType.bypass,
                replica_groups=[list(range(NCORES))],
                ins=[h16sh[:]],
                outs=[h16full[:]],
            )

            # ---- layer 2: h16full -> out
            layer(h16full, HID_DIM, w2_sb, OUT_DIM, b2_sb, False, store(out_d, mybir.dt.float32, OUT_DIM, False))

    nc.compile()
    return nc


_CACHE = {}


def _enable_trace_shim():
    """This image's antenv lacks axon_hooks; recreate it so trace=True works,
    and stub the artifact upload (no bucket access here)."""
    import types

    try:
        import antenv.axon_hooks  # noqa: F401
    except ImportError:
        mod = types.ModuleType("antenv.axon_hooks")
        _h = [None]
        mod.set_axon_ntff_profile_hook = lambda h: _h.__setitem__(0, h)
        mod.get_axon_ntff_profile_hook = lambda: _h[0]
        sys.modules["antenv.axon_hooks"] = mod
        import antenv

        antenv.axon_hooks = mod
        from trn_agent_boot.trn_boot import _ntff_profile_via_ctypes

        mod.set_axon_ntff_profile_hook(
            _ntff_profile_via_ctypes("/opt/axon/libaxon_pjrt.so")
        )
    import concourse.bass_utils as bu

    bu.upload_artifacts = lambda tmpdir: tmpdir


def kernel(x, edge_index, W1, b1, W2, b2):
    global LAST_RESULTS
    meta, in_maps = preprocess(x, edge_index, W1, b1, W2, b2)
    key = (tuple(meta["cap"].reshape(-1)), meta["has_b1"], meta["has_b2"])
    if key not in _CACHE:
        _CACHE[key] = build(meta)
    nc = _CACHE[key]
    trace = bool(int(os.environ.get("GCN_TRACE", "0")))
    if trace:
        _enable_trace_shim()
    res = run_bass_kernel_spmd(
        nc, in_maps, core_ids=list(range(NCORES)), trace=trace
    )
    LAST_RESULTS = res
    return np.concatenate([res.results[c]["out"] for c in range(NCORES)], axis=0)


# revision 12
# speedup vs baseline: 1.1253x; 1.1253x over previous
"""Two-layer GCN on 8 Trainium2 NeuronCores (Bass/Tile).

Math (reference, per layer):
    deg  = segment_sum(ones, dst)                 # target-side degrees
    dinv = where(deg>0, rsqrt(deg), 0)
    out[d] = dinv[d] * sum_{e: dst[e]=d} dinv[src[e]] * x[src[e]]  @ W  + b
(the x@W GEMM commutes with the segment sum, so we aggregate raw features
and apply W once per 128-node output block).

Distribution: dst nodes (and their incident edges) are sharded across the 8
cores; x (fp16) is replicated in every core's HBM.  Per 128-edge chunk the
kernel gathers the source rows with dma_gather, builds a dinv-weighted
one-hot selection matrix [128e x 128dst] on DVE, and scatter-adds via a
TensorE matmul accumulating in PSUM: psum[f, d] += gathered.T @ sel.  A
second matmul applies the layer weight; the layer-1 activations are
exchanged with an AllGather so layer 2 can gather any source row.

dma_gather indices are int16, so sources are split into lo (< 32768) and
hi (>= 32768) edge lists; the hi gather reads from a view of the feature
table offset by 32768 rows.
"""

import os
import sys
import time

sys.path.insert(0, "/opt/trn_rl_repo")

import numpy as np

import concourse.bass as bass
import concourse.bacc as bacc
import concourse.tile as tile
from concourse import mybir
from concourse.bass_utils import run_bass_kernel_spmd

P = 128
N_NODES = 50000
N_EDGES = 800000
IN_DIM = 128
HID_DIM = 128
OUT_DIM = 64
NCORES = 8
SHARD = N_NODES // NCORES          # 6250
NBLK = (SHARD + P - 1) // P        # 49 dst blocks per core (48 full + 106)
SPLIT = 32768                      # int16 index limit
SB_BLOCKS = 7                      # dst blocks per superblock (gather batch)
LAST_ROWS = SHARD - (NBLK - 1) * P # rows in the final dst block

# Filled by kernel() on the last run (for test.py introspection).
LAST_RESULTS = None


# --------------------------------------------------------------------------
# Host-side preprocessing
# --------------------------------------------------------------------------

def _wrap_idx(idx_chunks):
    """int16 indices for one dma_gather call: [16, n/16] wrap replicated to
    128 partitions.  idx_chunks: int array [n_chunks, 128]."""
    flat = idx_chunks.reshape(-1)
    n = flat.shape[0]
    arr = flat.reshape(n // 16, 16).T.astype(np.int16)   # [16, n/16]
    return np.tile(arr, (8, 1))                          # [128, n/16]


def preprocess(x, edge_index, W1, b1, W2, b2):
    x = np.asarray(x, dtype=np.float32)
    edge_index = np.asarray(edge_index).astype(np.int64)
    src_g = edge_index[0].astype(np.int32)
    dst_g = edge_index[1].astype(np.int32)

    deg = np.bincount(dst_g, minlength=N_NODES).astype(np.float32)
    dinv = np.where(deg > 0, 1.0 / np.sqrt(np.maximum(deg, 1.0)), 0.0).astype(
        np.float32
    )

    # per (core, block, lo/hi) edge lists
    owner = dst_g // SHARD
    blk_loc = (dst_g % SHARD) // P
    rel = (dst_g % SHARD) % P
    is_lo = src_g < SPLIT

    # bucket sort edges by (owner, block, hi/lo)
    key = ((owner * NBLK + blk_loc) * 2 + (~is_lo).astype(np.int32)).astype(np.int64)
    order = np.argsort(key, kind="stable")
    key_s = key[order]
    src_s = src_g[order]
    rel_s = rel[order]
    bounds = np.searchsorted(key_s, np.arange(NCORES * NBLK * 2 + 1))

    def bucket(c, b, hi):
        k = (c * NBLK + b) * 2 + hi
        lo_i, hi_i = bounds[k], bounds[k + 1]
        return src_s[lo_i:hi_i], rel_s[lo_i:hi_i]

    nchunks = np.zeros((NCORES, NBLK, 2), np.int64)
    for c in range(NCORES):
        for b in range(NBLK):
            for h in (0, 1):
                n = bounds[(c * NBLK + b) * 2 + h + 1] - bounds[(c * NBLK + b) * 2 + h]
                nchunks[c, b, h] = -(-n // P)
    cap = nchunks.max(axis=0)                       # [NBLK, 2] shared structure
    cap[:, 0] = np.maximum(cap[:, 0], 1)            # >=1 chunk per block

    # superblock structure
    sbs = [list(range(s, min(s + SB_BLOCKS, NBLK))) for s in range(0, NBLK, SB_BLOCKS)]
    meta = {
        "cap": cap,
        "sbs": sbs,
        "has_b1": bool(np.any(np.asarray(b1))),
        "has_b2": bool(np.any(np.asarray(b2))),
    }

    # per-core arrays.  x is pre-scaled by dinv[src] so the selection matrix
    # is a plain one-hot (single DVE op); padded lanes get dstrel=-1, which
    # matches no iota column and therefore contributes nothing.
    total_lo = int(cap[:, 0].sum())
    total_hi = int(cap[:, 1].sum())
    in_maps = []
    x16 = (dinv[:, None] * x).astype(np.float16)
    for c in range(NCORES):
        idx_lo = np.zeros((total_lo, P), np.int32)
        idx_hi = np.zeros((total_hi, P), np.int32)
        m_dst = np.zeros((P, total_lo + total_hi), np.float32)
        off = {0: 0, 1: 0}
        for b in range(NBLK):
            for h in (0, 1):
                s_arr, r_arr = bucket(c, b, h)
                n = s_arr.shape[0]
                ncap = int(cap[b, h])
                idxs = np.zeros(ncap * P, np.int32)
                idxs[:n] = s_arr - (SPLIT if h else 0)
                d_arr = np.full(ncap * P, -1.0, np.float32)
                d_arr[:n] = r_arr
                o = off[h]
                tgt = idx_lo if h == 0 else idx_hi
                tgt[o : o + ncap] = idxs.reshape(ncap, P)
                col0 = o if h == 0 else total_lo + o
                m_dst[:, col0 : col0 + ncap] = d_arr.reshape(ncap, P).T
                off[h] += ncap

        # wrap gather indices per superblock call
        lo_cols, hi_cols = [], []
        lo_o = hi_o = 0
        for sb in sbs:
            n_lo = int(cap[sb, 0].sum())
            n_hi = int(cap[sb, 1].sum())
            lo_cols.append(_wrap_idx(idx_lo[lo_o : lo_o + n_lo]))
            lo_o += n_lo
            if n_hi:
                hi_cols.append(_wrap_idx(idx_hi[hi_o : hi_o + n_hi]))
                hi_o += n_hi
        idx_lo_w = np.concatenate(lo_cols, axis=1)
        idx_hi_w = (
            np.concatenate(hi_cols, axis=1)
            if hi_cols
            else np.zeros((P, 8), np.int16)
        )

        tmp = np.zeros(NBLK * P, np.float32)
        tmp[:SHARD] = dinv[c * SHARD : (c + 1) * SHARD]
        dinvd = tmp.reshape(NBLK, P).T.copy()   # [p, b] = dinv[c*SHARD + b*P + p]

        im = {
            "x16": x16,
            "idx_lo": idx_lo_w,
            "idx_hi": idx_hi_w,
            "m_dst": m_dst,
            "m_dstn": -m_dst,
            "dinvd": dinvd,
            "dinvd2": dinvd * dinvd,
            "w1": np.asarray(W1, np.float32).astype(np.float16),
            "w2": np.asarray(W2, np.float32).astype(np.float16),
        }
        if meta["has_b1"]:
            im["b1rep"] = np.tile(np.asarray(b1, np.float32)[None, :], (P, 1))
        if meta["has_b2"]:
            im["b2rep"] = np.tile(np.asarray(b2, np.float32)[None, :], (P, 1))
        in_maps.append(im)
    return meta, in_maps


# --------------------------------------------------------------------------
# Bass kernel
# --------------------------------------------------------------------------

def build(meta):
    cap = meta["cap"]
    sbs = meta["sbs"]
    total_lo = int(cap[:, 0].sum())
    total_hi = int(cap[:, 1].sum())

    nc = bacc.Bacc(
        "TRN2",
        target_bir_lowering=False,
        debug=False,
        enable_asserts=True,
        num_devices=NCORES,
        num_swdge_queues=4,
    )
    x16 = nc.dram_tensor("x16", [N_NODES, IN_DIM], mybir.dt.float16, kind="ExternalInput")
    idx_lo_d = nc.dram_tensor("idx_lo", [P, total_lo * 8], mybir.dt.int16, kind="ExternalInput")
    idx_hi_d = nc.dram_tensor(
        "idx_hi", [P, max(total_hi, 1) * 8], mybir.dt.int16, kind="ExternalInput"
    )
    m_dst_d = nc.dram_tensor("m_dst", [P, total_lo + total_hi], mybir.dt.float32, kind="ExternalInput")
    m_dstn_d = nc.dram_tensor("m_dstn", [P, total_lo + total_hi], mybir.dt.float32, kind="ExternalInput")
    dinvd_d = nc.dram_tensor("dinvd", [P, NBLK], mybir.dt.float32, kind="ExternalInput")
    dinvd2_d = nc.dram_tensor("dinvd2", [P, NBLK], mybir.dt.float32, kind="ExternalInput")
    w1_d = nc.dram_tensor("w1", [IN_DIM, HID_DIM], mybir.dt.float16, kind="ExternalInput")
    w2_d = nc.dram_tensor("w2", [HID_DIM, OUT_DIM], mybir.dt.float16, kind="ExternalInput")
    b1_d = (
        nc.dram_tensor("b1rep", [P, HID_DIM], mybir.dt.float32, kind="ExternalInput")
        if meta["has_b1"]
        else None
    )
    b2_d = (
        nc.dram_tensor("b2rep", [P, OUT_DIM], mybir.dt.float32, kind="ExternalInput")
        if meta["has_b2"]
        else None
    )
    out_d = nc.dram_tensor("out", [SHARD, OUT_DIM], mybir.dt.float32, kind="ExternalOutput")

    with tile.TileContext(nc) as tc:
        with (
            tc.tile_pool(name="const", bufs=1) as const,
            tc.tile_pool(name="gpool", bufs=2) as gpool,
            tc.tile_pool(name="selp", bufs=16) as selp,
            tc.tile_pool(name="sbuf", bufs=3) as sbp,
            tc.tile_pool(name="psA", bufs=4, space="PSUM") as psA,
            tc.tile_pool(name="psB", bufs=2, space="PSUM") as psB,
            tc.tile_pool(name="dram", bufs=1, space="DRAM") as dram,
        ):
            # ---- one-time loads
            idx_lo_sb = const.tile([P, total_lo * 8], mybir.dt.int16)
            nc.sync.dma_start(out=idx_lo_sb[:], in_=idx_lo_d[:])
            idx_hi_sb = const.tile([P, max(total_hi, 1) * 8], mybir.dt.int16)
            nc.sync.dma_start(out=idx_hi_sb[:], in_=idx_hi_d[:])
            m_dst_sb = const.tile([P, total_lo + total_hi], mybir.dt.float32)
            nc.sync.dma_start(out=m_dst_sb[:], in_=m_dst_d[:])
            m_dstn_sb = const.tile([P, total_lo + total_hi], mybir.dt.float32)
            nc.sync.dma_start(out=m_dstn_sb[:], in_=m_dstn_d[:])
            dinvd_sb = const.tile([P, NBLK], mybir.dt.float32)
            nc.sync.dma_start(out=dinvd_sb[:], in_=dinvd_d[:])
            dinvd2_sb = const.tile([P, NBLK], mybir.dt.float32)
            nc.sync.dma_start(out=dinvd2_sb[:], in_=dinvd2_d[:])
            w1_sb = const.tile([IN_DIM, HID_DIM], mybir.dt.float16)
            nc.sync.dma_start(out=w1_sb[:], in_=w1_d[:])
            w2_sb = const.tile([HID_DIM, OUT_DIM], mybir.dt.float16)
            nc.sync.dma_start(out=w2_sb[:], in_=w2_d[:])
            b1_sb = b2_sb = None
            if b1_d is not None:
                b1_sb = const.tile([P, HID_DIM], mybir.dt.float32)
                nc.sync.dma_start(out=b1_sb[:], in_=b1_d[:])
            if b2_d is not None:
                b2_sb = const.tile([P, OUT_DIM], mybir.dt.float32)
                nc.sync.dma_start(out=b2_sb[:], in_=b2_d[:])

            iota32 = const.tile([P, P], mybir.dt.int32)
            nc.gpsimd.iota(iota32[:], pattern=[[1, P]], base=0, channel_multiplier=0)
            iota16 = const.tile([P, P], mybir.dt.float16)
            nc.vector.tensor_copy(out=iota16[:], in_=iota32[:])
            iota32f = const.tile([P, P], mybir.dt.float32)
            nc.vector.tensor_copy(out=iota32f[:], in_=iota32[:])

            h16sh = dram.tile([SHARD, HID_DIM], mybir.dt.float16)
            h16full = dram.tile([N_NODES, HID_DIM], mybir.dt.float16, addr_space="Shared")

            # SWDGE descriptor rings can't hold a whole-superblock gather in
            # one instruction (ring carveout is O(512) descs/engine; the
            # ucode's await_space never succeeds past that) — split calls.
            MAXCH = int(os.environ.get("GCN_GATHER_CHUNKS", "16"))
            qrot = [0]

            def gather_split(dst_tile, src_ap, idx_sb, ch_off, n_ch, fin):
                for k0 in range(0, n_ch, MAXCH):
                    kn = min(MAXCH, n_ch - k0)
                    nc.gpsimd.dma_gather(
                        out_ap=dst_tile[:, k0 : k0 + kn, :],
                        in_ap=src_ap,
                        idxs_ap=idx_sb[:, (ch_off + k0) * 8 : (ch_off + k0 + kn) * 8],
                        num_idxs=kn * P,
                        num_idxs_reg=kn * P,
                        elem_size=fin,
                        single_packet=False,
                        queue_num=qrot[0] % 4,
                    )
                    qrot[0] += 1

            def layer(src_dram, fin, w_sb, fout, bias_sb, relu, sink):
                lo_off = 0          # lo chunk offset (also m_dst/m_w column)
                hi_off = 0
                for sb in sbs:
                    n_lo = int(cap[sb, 0].sum())
                    n_hi = int(cap[sb, 1].sum())
                    glo = gpool.tile([P, n_lo, fin], mybir.dt.float16, tag="glo")
                    gather_split(glo, src_dram[:], idx_lo_sb, lo_off, n_lo, fin)
                    ghi = None
                    if n_hi:
                        ghi = gpool.tile([P, n_hi, fin], mybir.dt.float16, tag="ghi")
                        gather_split(ghi, src_dram[SPLIT:, :], idx_hi_sb, hi_off, n_hi, fin)
                    lo_c = 0
                    hi_c = 0
                    for b in sb:
                        # chunk list for this block: (tile, col in tile, meta col)
                        chunks = []
                        for k in range(int(cap[b, 0])):
                            chunks.append((glo, lo_c + k, lo_off + lo_c + k))
                        for k in range(int(cap[b, 1])):
                            chunks.append(
                                (ghi, hi_c + k, total_lo + hi_off + hi_c + k)
                            )
                        lo_c += int(cap[b, 0])
                        hi_c += int(cap[b, 1])

                        ps_s = psA.tile([P, P], mybir.dt.float32, tag="psA")
                        for j, (gt, gc, mc) in enumerate(chunks):
                            sel = selp.tile([P, P], mybir.dt.float16, tag="sel")
                            nc.vector.tensor_scalar(
                                out=sel[:],
                                in0=iota16[:],
                                scalar1=m_dst_sb[:, mc : mc + 1],
                                scalar2=None,
                                op0=mybir.AluOpType.is_equal,
                            )
                            nc.tensor.matmul(
                                out=ps_s[:],
                                lhsT=gt[:, gc, :],
                                rhs=sel[:],
                                start=(j == 0),
                                stop=(j == len(chunks) - 1),
                            )
                        sT = sbp.tile([P, P], mybir.dt.float16, tag="sT")
                        nc.vector.tensor_copy(out=sT[:], in_=ps_s[:])
                        ps_h = psB.tile([P, fout], mybir.dt.float32, tag="psB")
                        nc.tensor.matmul(
                            out=ps_h[:], lhsT=sT[:], rhs=w_sb[:], start=True, stop=True
                        )
                        sink(b, ps_h, bias_sb, relu)
                    lo_off += n_lo
                    hi_off += n_hi

            def store(dst_dram, dt, fout, extra_dinv):
                # layer 1 stores h16' = dinv * relu(dinv*z + b1) (the leading
                # dinv is the src-side prescale for layer 2's gather); with
                # b1 == 0 this folds to relu(dinv^2 * z) in one ACT op.
                def sink(b, ps_h, bias_sb, relu):
                    rows = P if b < NBLK - 1 else LAST_ROWS
                    o_t = sbp.tile([P, fout], dt, tag=f"o{dt}")
                    if bias_sb is None:
                        sc = dinvd2_sb if extra_dinv else dinvd_sb
                        nc.scalar.activation(
                            out=o_t[:],
                            in_=ps_h[:],
                            func=(
                                mybir.ActivationFunctionType.Relu
                                if relu
                                else mybir.ActivationFunctionType.Copy
                            ),
                            scale=sc[:, b : b + 1],
                        )
                    else:
                        t1 = sbp.tile([P, fout], mybir.dt.float32, tag="t1")
                        nc.vector.tensor_scalar(
                            out=t1[:],
                            in0=ps_h[:],
                            scalar1=dinvd_sb[:, b : b + 1],
                            scalar2=None,
                            op0=mybir.AluOpType.mult,
                        )
                        nc.vector.tensor_tensor(
                            out=t1[:], in0=t1[:], in1=bias_sb[:], op=mybir.AluOpType.add
                        )
                        if relu:
                            nc.scalar.activation(
                                out=o_t[:],
                                in_=t1[:],
                                func=mybir.ActivationFunctionType.Relu,
                                scale=(
                                    dinvd_sb[:, b : b + 1] if extra_dinv else 1.0
                                ),
                            )
                        elif extra_dinv:
                            nc.vector.tensor_scalar(
                                out=o_t[:],
                                in0=t1[:],
                                scalar1=dinvd_sb[:, b : b + 1],
                                scalar2=None,
                                op0=mybir.AluOpType.mult,
                            )
                        else:
                            nc.vector.tensor_copy(out=o_t[:], in_=t1[:])
                    nc.sync.dma_start(
                        out=dst_dram[b * P : b * P + rows, :], in_=o_t[:rows, :]
                    )

                return sink

            # ---- layer 1: x16 -> h16sh
            layer(x16, IN_DIM, w1_sb, HID_DIM, b1_sb, True, store(h16sh, mybir.dt.float16, HID_DIM, True))

            # ---- exchange
            nc.gpsimd.collective_compute(
                "AllGather",
                mybir.AluOpType.bypass,
                replica_groups=[list(range(NCORES))],
                ins=[h16sh[:]],
                outs=[h16full[:]],
            )

            # ---- layer 2: h16full -> out
            layer(h16full, HID_DIM, w2_sb, OUT_DIM, b2_sb, False, store(out_d, mybir.dt.float32, OUT_DIM, False))

    nc.compile()
    return nc


_CACHE = {}


def _enable_trace_shim():
    """This image's antenv lacks axon_hooks; recreate it so trace=True works,
    and stub the artifact upload (no bucket access here)."""
    import types

    try:
        import antenv.axon_hooks  # noqa: F401
    except ImportError:
        mod = types.ModuleType("antenv.axon_hooks")
        _h = [None]
        mod.set_axon_ntff_profile_hook = lambda h: _h.__setitem__(0, h)
        mod.get_axon_ntff_profile_hook = lambda: _h[0]
        sys.modules["antenv.axon_hooks"] = mod
        import antenv

        antenv.axon_hooks = mod
        from trn_agent_boot.trn_boot import _ntff_profile_via_ctypes

        mod.set_axon_ntff_profile_hook(
            _ntff_profile_via_ctypes("/opt/axon/libaxon_pjrt.so")
        )
    import concourse.bass_utils as bu

    bu.upload_artifacts = lambda tmpdir: tmpdir


def kernel(x, edge_index, W1, b1, W2, b2):
    global LAST_RESULTS
    meta, in_maps = preprocess(x, edge_index, W1, b1, W2, b2)
    key = (tuple(meta["cap"].reshape(-1)), meta["has_b1"], meta["has_b2"])
    if key not in _CACHE:
        _CACHE[key] = build(meta)
    nc = _CACHE[key]
    trace = bool(int(os.environ.get("GCN_TRACE", "0")))
    if trace:
        _enable_trace_shim()
    res = run_bass_kernel_spmd(
        nc, in_maps, core_ids=list(range(NCORES)), trace=trace
    )
    LAST_RESULTS = res
    return np.concatenate([res.results[c]["out"] for c in range(NCORES)], axis=0)


# revision 13
# speedup vs baseline: 1.3536x; 1.2029x over previous
"""Two-layer GCN on 8 Trainium2 NeuronCores (Bass/Tile).

Math (reference, per layer):
    deg  = segment_sum(ones, dst)                 # target-side degrees
    dinv = where(deg>0, rsqrt(deg), 0)
    out[d] = dinv[d] * sum_{e: dst[e]=d} dinv[src[e]] * x[src[e]]  @ W  + b
(the x@W GEMM commutes with the segment sum, so we aggregate raw features
and apply W once per 128-node output block).

Distribution: dst nodes (and their incident edges) are sharded across the 8
cores; x (fp16) is replicated in every core's HBM.  Per 128-edge chunk the
kernel gathers the source rows with dma_gather, builds a dinv-weighted
one-hot selection matrix [128e x 128dst] on DVE, and scatter-adds via a
TensorE matmul accumulating in PSUM: psum[f, d] += gathered.T @ sel.  A
second matmul applies the layer weight; the layer-1 activations are
exchanged with an AllGather so layer 2 can gather any source row.

dma_gather indices are int16, so sources are split into lo (< 32768) and
hi (>= 32768) edge lists; the hi gather reads from a view of the feature
table offset by 32768 rows.
"""

import os
import sys
import time

sys.path.insert(0, "/opt/trn_rl_repo")

import numpy as np

import concourse.bass as bass
import concourse.bacc as bacc
import concourse.tile as tile
from concourse import mybir
from concourse.bass_utils import run_bass_kernel_spmd

P = 128
N_NODES = 50000
N_EDGES = 800000
IN_DIM = 128
HID_DIM = 128
OUT_DIM = 64
NCORES = 8
SHARD = N_NODES // NCORES          # 6250
NBLK = (SHARD + P - 1) // P        # 49 dst blocks per core (48 full + 106)
SPLIT = 32768                      # int16 index limit
SB_BLOCKS = 7                      # dst blocks per superblock (gather batch)
LAST_ROWS = SHARD - (NBLK - 1) * P # rows in the final dst block

# Filled by kernel() on the last run (for test.py introspection).
LAST_RESULTS = None


# --------------------------------------------------------------------------
# Host-side preprocessing
# --------------------------------------------------------------------------

def _wrap_idx(idx_chunks):
    """int16 indices for one dma_gather call: [16, n/16] wrap replicated to
    128 partitions.  idx_chunks: int array [n_chunks, 128]."""
    flat = idx_chunks.reshape(-1)
    n = flat.shape[0]
    arr = flat.reshape(n // 16, 16).T.astype(np.int16)   # [16, n/16]
    return np.tile(arr, (8, 1))                          # [128, n/16]


def preprocess(x, edge_index, W1, b1, W2, b2):
    x = np.asarray(x, dtype=np.float32)
    edge_index = np.asarray(edge_index).astype(np.int64)
    src_g = edge_index[0].astype(np.int32)
    dst_g = edge_index[1].astype(np.int32)

    deg = np.bincount(dst_g, minlength=N_NODES).astype(np.float32)
    dinv = np.where(deg > 0, 1.0 / np.sqrt(np.maximum(deg, 1.0)), 0.0).astype(
        np.float32
    )

    # per (core, block, lo/hi) edge lists
    owner = dst_g // SHARD
    blk_loc = (dst_g % SHARD) // P
    rel = (dst_g % SHARD) % P
    is_lo = src_g < SPLIT

    # bucket sort edges by (owner, block, hi/lo)
    key = ((owner * NBLK + blk_loc) * 2 + (~is_lo).astype(np.int32)).astype(np.int64)
    order = np.argsort(key, kind="stable")
    key_s = key[order]
    src_s = src_g[order]
    rel_s = rel[order]
    bounds = np.searchsorted(key_s, np.arange(NCORES * NBLK * 2 + 1))

    def bucket(c, b, hi):
        k = (c * NBLK + b) * 2 + hi
        lo_i, hi_i = bounds[k], bounds[k + 1]
        return src_s[lo_i:hi_i], rel_s[lo_i:hi_i]

    nchunks = np.zeros((NCORES, NBLK, 2), np.int64)
    for c in range(NCORES):
        for b in range(NBLK):
            for h in (0, 1):
                n = bounds[(c * NBLK + b) * 2 + h + 1] - bounds[(c * NBLK + b) * 2 + h]
                nchunks[c, b, h] = -(-n // P)
    cap = nchunks.max(axis=0)                       # [NBLK, 2] shared structure
    cap[:, 0] = np.maximum(cap[:, 0], 1)            # >=1 chunk per block

    # superblock structure
    sbs = [list(range(s, min(s + SB_BLOCKS, NBLK))) for s in range(0, NBLK, SB_BLOCKS)]
    meta = {
        "cap": cap,
        "sbs": sbs,
        "has_b1": bool(np.any(np.asarray(b1))),
        "has_b2": bool(np.any(np.asarray(b2))),
    }

    # per-core arrays.  x is pre-scaled by dinv[src] so the selection matrix
    # is a plain one-hot (single DVE op); padded lanes get dstrel=-1, which
    # matches no iota column and therefore contributes nothing.
    total_lo = int(cap[:, 0].sum())
    total_hi = int(cap[:, 1].sum())
    in_maps = []
    x16 = (dinv[:, None] * x).astype(np.float16)
    for c in range(NCORES):
        idx_lo = np.zeros((total_lo, P), np.int32)
        idx_hi = np.zeros((total_hi, P), np.int32)
        m_dst = np.zeros((P, total_lo + total_hi), np.float32)
        off = {0: 0, 1: 0}
        for b in range(NBLK):
            for h in (0, 1):
                s_arr, r_arr = bucket(c, b, h)
                n = s_arr.shape[0]
                ncap = int(cap[b, h])
                idxs = np.zeros(ncap * P, np.int32)
                idxs[:n] = s_arr - (SPLIT if h else 0)
                d_arr = np.full(ncap * P, -1.0, np.float32)
                d_arr[:n] = r_arr
                o = off[h]
                tgt = idx_lo if h == 0 else idx_hi
                tgt[o : o + ncap] = idxs.reshape(ncap, P)
                col0 = o if h == 0 else total_lo + o
                m_dst[:, col0 : col0 + ncap] = d_arr.reshape(ncap, P).T
                off[h] += ncap

        # wrap gather indices per superblock call
        lo_cols, hi_cols = [], []
        lo_o = hi_o = 0
        for sb in sbs:
            n_lo = int(cap[sb, 0].sum())
            n_hi = int(cap[sb, 1].sum())
            lo_cols.append(_wrap_idx(idx_lo[lo_o : lo_o + n_lo]))
            lo_o += n_lo
            if n_hi:
                hi_cols.append(_wrap_idx(idx_hi[hi_o : hi_o + n_hi]))
                hi_o += n_hi
        idx_lo_w = np.concatenate(lo_cols, axis=1)
        idx_hi_w = (
            np.concatenate(hi_cols, axis=1)
            if hi_cols
            else np.zeros((P, 8), np.int16)
        )

        tmp = np.zeros(NBLK * P, np.float32)
        tmp[:SHARD] = dinv[c * SHARD : (c + 1) * SHARD]
        dinvd = tmp.reshape(NBLK, P).T.copy()   # [p, b] = dinv[c*SHARD + b*P + p]

        im = {
            "x16": x16,
            "idx_lo": idx_lo_w,
            "idx_hi": idx_hi_w,
            "m_dst16": m_dst.astype(np.float16),
            "dinvd": dinvd,
            "dinvd2": dinvd * dinvd,
            "w1": np.asarray(W1, np.float32).astype(np.float16),
            "w2": np.asarray(W2, np.float32).astype(np.float16),
        }
        if meta["has_b1"]:
            im["b1rep"] = np.tile(np.asarray(b1, np.float32)[None, :], (P, 1))
        if meta["has_b2"]:
            im["b2rep"] = np.tile(np.asarray(b2, np.float32)[None, :], (P, 1))
        in_maps.append(im)
    return meta, in_maps


# --------------------------------------------------------------------------
# Bass kernel
# --------------------------------------------------------------------------

def build(meta):
    cap = meta["cap"]
    sbs = meta["sbs"]
    total_lo = int(cap[:, 0].sum())
    total_hi = int(cap[:, 1].sum())

    nc = bacc.Bacc(
        "TRN2",
        target_bir_lowering=False,
        debug=False,
        enable_asserts=True,
        num_devices=NCORES,
        num_swdge_queues=4,
    )
    x16 = nc.dram_tensor("x16", [N_NODES, IN_DIM], mybir.dt.float16, kind="ExternalInput")
    idx_lo_d = nc.dram_tensor("idx_lo", [P, total_lo * 8], mybir.dt.int16, kind="ExternalInput")
    idx_hi_d = nc.dram_tensor(
        "idx_hi", [P, max(total_hi, 1) * 8], mybir.dt.int16, kind="ExternalInput"
    )
    m_dst16_d = nc.dram_tensor("m_dst16", [P, total_lo + total_hi], mybir.dt.float16, kind="ExternalInput")
    dinvd_d = nc.dram_tensor("dinvd", [P, NBLK], mybir.dt.float32, kind="ExternalInput")
    dinvd2_d = nc.dram_tensor("dinvd2", [P, NBLK], mybir.dt.float32, kind="ExternalInput")
    w1_d = nc.dram_tensor("w1", [IN_DIM, HID_DIM], mybir.dt.float16, kind="ExternalInput")
    w2_d = nc.dram_tensor("w2", [HID_DIM, OUT_DIM], mybir.dt.float16, kind="ExternalInput")
    b1_d = (
        nc.dram_tensor("b1rep", [P, HID_DIM], mybir.dt.float32, kind="ExternalInput")
        if meta["has_b1"]
        else None
    )
    b2_d = (
        nc.dram_tensor("b2rep", [P, OUT_DIM], mybir.dt.float32, kind="ExternalInput")
        if meta["has_b2"]
        else None
    )
    out_d = nc.dram_tensor("out", [SHARD, OUT_DIM], mybir.dt.float32, kind="ExternalOutput")

    with tile.TileContext(nc) as tc:
        with (
            tc.tile_pool(name="const", bufs=1) as const,
            tc.tile_pool(name="gpool", bufs=2) as gpool,
            tc.tile_pool(name="selp", bufs=8) as selp,
            tc.tile_pool(name="sbuf", bufs=3) as sbp,
            tc.tile_pool(name="psA", bufs=4, space="PSUM") as psA,
            tc.tile_pool(name="psB", bufs=2, space="PSUM") as psB,
            tc.tile_pool(name="dram", bufs=1, space="DRAM") as dram,
        ):
            # ---- one-time loads
            idx_lo_sb = const.tile([P, total_lo * 8], mybir.dt.int16)
            nc.sync.dma_start(out=idx_lo_sb[:], in_=idx_lo_d[:])
            idx_hi_sb = const.tile([P, max(total_hi, 1) * 8], mybir.dt.int16)
            nc.sync.dma_start(out=idx_hi_sb[:], in_=idx_hi_d[:])
            m_dst16_sb = const.tile([P, total_lo + total_hi], mybir.dt.float16)
            nc.sync.dma_start(out=m_dst16_sb[:], in_=m_dst16_d[:])
            dinvd_sb = const.tile([P, NBLK], mybir.dt.float32)
            nc.sync.dma_start(out=dinvd_sb[:], in_=dinvd_d[:])
            dinvd2_sb = const.tile([P, NBLK], mybir.dt.float32)
            nc.sync.dma_start(out=dinvd2_sb[:], in_=dinvd2_d[:])
            w1_sb = const.tile([IN_DIM, HID_DIM], mybir.dt.float16)
            nc.sync.dma_start(out=w1_sb[:], in_=w1_d[:])
            w2_sb = const.tile([HID_DIM, OUT_DIM], mybir.dt.float16)
            nc.sync.dma_start(out=w2_sb[:], in_=w2_d[:])
            b1_sb = b2_sb = None
            if b1_d is not None:
                b1_sb = const.tile([P, HID_DIM], mybir.dt.float32)
                nc.sync.dma_start(out=b1_sb[:], in_=b1_d[:])
            if b2_d is not None:
                b2_sb = const.tile([P, OUT_DIM], mybir.dt.float32)
                nc.sync.dma_start(out=b2_sb[:], in_=b2_d[:])

            iota32 = const.tile([P, P], mybir.dt.int32)
            nc.gpsimd.iota(iota32[:], pattern=[[1, P]], base=0, channel_multiplier=0)
            iota16 = const.tile([P, P], mybir.dt.float16)
            nc.vector.tensor_copy(out=iota16[:], in_=iota32[:])
            BW = 8
            iota16b = const.tile([P, BW, P], mybir.dt.float16)
            for g in range(BW):
                nc.vector.tensor_copy(out=iota16b[:, g, :], in_=iota16[:])

            h16sh = dram.tile([SHARD, HID_DIM], mybir.dt.float16)
            h16full = dram.tile([N_NODES, HID_DIM], mybir.dt.float16, addr_space="Shared")

            # SWDGE descriptor rings can't hold a whole-superblock gather in
            # one instruction (ring carveout is O(512) descs/engine; the
            # ucode's await_space never succeeds past that) — split calls.
            MAXCH = int(os.environ.get("GCN_GATHER_CHUNKS", "16"))
            qrot = [0]

            def gather_split(dst_tile, src_ap, idx_sb, ch_off, n_ch, fin):
                for k0 in range(0, n_ch, MAXCH):
                    kn = min(MAXCH, n_ch - k0)
                    nc.gpsimd.dma_gather(
                        out_ap=dst_tile[:, k0 : k0 + kn, :],
                        in_ap=src_ap,
                        idxs_ap=idx_sb[:, (ch_off + k0) * 8 : (ch_off + k0 + kn) * 8],
                        num_idxs=kn * P,
                        num_idxs_reg=kn * P,
                        elem_size=fin,
                        single_packet=False,
                        queue_num=qrot[0] % 4,
                    )
                    qrot[0] += 1

            def layer(src_dram, fin, w_sb, fout, bias_sb, relu, sink):
                lo_off = 0          # lo chunk offset (also m_dst/m_w column)
                hi_off = 0
                for sb in sbs:
                    n_lo = int(cap[sb, 0].sum())
                    n_hi = int(cap[sb, 1].sum())
                    glo = gpool.tile([P, n_lo, fin], mybir.dt.float16, tag="glo")
                    gather_split(glo, src_dram[:], idx_lo_sb, lo_off, n_lo, fin)
                    ghi = None
                    if n_hi:
                        ghi = gpool.tile([P, n_hi, fin], mybir.dt.float16, tag="ghi")
                        gather_split(ghi, src_dram[SPLIT:, :], idx_hi_sb, hi_off, n_hi, fin)
                    lo_c = 0
                    hi_c = 0
                    for b in sb:
                        # two contiguous chunk runs per block (lo then hi)
                        runs = []
                        if int(cap[b, 0]):
                            runs.append((glo, lo_c, lo_off + lo_c, int(cap[b, 0])))
                        if int(cap[b, 1]):
                            runs.append(
                                (ghi, hi_c, total_lo + hi_off + hi_c, int(cap[b, 1]))
                            )
                        lo_c += int(cap[b, 0])
                        hi_c += int(cap[b, 1])
                        total = sum(r[3] for r in runs)

                        ps_s = psA.tile([P, P], mybir.dt.float32, tag="psA")
                        jj = 0
                        for gt, gc0, mc0, cnt in runs:
                            for g0 in range(0, cnt, BW):
                                g = min(BW, cnt - g0)
                                selt = selp.tile([P, BW, P], mybir.dt.float16, tag="selb")
                                nc.vector.tensor_tensor(
                                    out=selt[:, :g, :],
                                    in0=m_dst16_sb[
                                        :, mc0 + g0 : mc0 + g0 + g
                                    ].to_broadcast([P, g, P]),
                                    in1=iota16b[:, :g, :],
                                    op=mybir.AluOpType.is_equal,
                                )
                                for k in range(g):
                                    nc.tensor.matmul(
                                        out=ps_s[:],
                                        lhsT=gt[:, gc0 + g0 + k, :],
                                        rhs=selt[:, k, :],
                                        start=(jj == 0),
                                        stop=(jj == total - 1),
                                    )
                                    jj += 1
                        sT = sbp.tile([P, P], mybir.dt.float16, tag="sT")
                        nc.vector.tensor_copy(out=sT[:], in_=ps_s[:])
                        ps_h = psB.tile([P, fout], mybir.dt.float32, tag="psB")
                        nc.tensor.matmul(
                            out=ps_h[:], lhsT=sT[:], rhs=w_sb[:], start=True, stop=True
                        )
                        sink(b, ps_h, bias_sb, relu)
                    lo_off += n_lo
                    hi_off += n_hi

            def store(dst_dram, dt, fout, extra_dinv):
                # layer 1 stores h16' = dinv * relu(dinv*z + b1) (the leading
                # dinv is the src-side prescale for layer 2's gather); with
                # b1 == 0 this folds to relu(dinv^2 * z) in one ACT op.
                def sink(b, ps_h, bias_sb, relu):
                    rows = P if b < NBLK - 1 else LAST_ROWS
                    o_t = sbp.tile([P, fout], dt, tag=f"o{dt}")
                    if bias_sb is None:
                        sc = dinvd2_sb if extra_dinv else dinvd_sb
                        nc.scalar.activation(
                            out=o_t[:],
                            in_=ps_h[:],
                            func=(
                                mybir.ActivationFunctionType.Relu
                                if relu
                                else mybir.ActivationFunctionType.Copy
                            ),
                            scale=sc[:, b : b + 1],
                        )
                    else:
                        t1 = sbp.tile([P, fout], mybir.dt.float32, tag="t1")
                        nc.vector.tensor_scalar(
                            out=t1[:],
                            in0=ps_h[:],
                            scalar1=dinvd_sb[:, b : b + 1],
                            scalar2=None,
                            op0=mybir.AluOpType.mult,
                        )
                        nc.vector.tensor_tensor(
                            out=t1[:], in0=t1[:], in1=bias_sb[:], op=mybir.AluOpType.add
                        )
                        if relu:
                            nc.scalar.activation(
                                out=o_t[:],
                                in_=t1[:],
                                func=mybir.ActivationFunctionType.Relu,
                                scale=(
                                    dinvd_sb[:, b : b + 1] if extra_dinv else 1.0
                                ),
                            )
                        elif extra_dinv:
                            nc.vector.tensor_scalar(
                                out=o_t[:],
                                in0=t1[:],
                                scalar1=dinvd_sb[:, b : b + 1],
                                scalar2=None,
                                op0=mybir.AluOpType.mult,
                            )
                        else:
                            nc.vector.tensor_copy(out=o_t[:], in_=t1[:])
                    nc.sync.dma_start(
                        out=dst_dram[b * P : b * P + rows, :], in_=o_t[:rows, :]
                    )

                return sink

            # ---- layer 1: x16 -> h16sh
            layer(x16, IN_DIM, w1_sb, HID_DIM, b1_sb, True, store(h16sh, mybir.dt.float16, HID_DIM, True))

            # ---- exchange
            nc.gpsimd.collective_compute(
                "AllGather",
                mybir.AluOpType.bypass,
                replica_groups=[list(range(NCORES))],
                ins=[h16sh[:]],
                outs=[h16full[:]],
            )

            # ---- layer 2: h16full -> out
            layer(h16full, HID_DIM, w2_sb, OUT_DIM, b2_sb, False, store(out_d, mybir.dt.float32, OUT_DIM, False))

    nc.compile()
    return nc


_CACHE = {}


def _enable_trace_shim():
    """This image's antenv lacks axon_hooks; recreate it so trace=True works,
    and stub the artifact upload (no bucket access here)."""
    import types

    try:
        import antenv.axon_hooks  # noqa: F401
    except ImportError:
        mod = types.ModuleType("antenv.axon_hooks")
        _h = [None]
        mod.set_axon_ntff_profile_hook = lambda h: _h.__setitem__(0, h)
        mod.get_axon_ntff_profile_hook = lambda: _h[0]
        sys.modules["antenv.axon_hooks"] = mod
        import antenv

        antenv.axon_hooks = mod
        from trn_agent_boot.trn_boot import _ntff_profile_via_ctypes

        mod.set_axon_ntff_profile_hook(
            _ntff_profile_via_ctypes("/opt/axon/libaxon_pjrt.so")
        )
    import concourse.bass_utils as bu

    bu.upload_artifacts = lambda tmpdir: tmpdir


def kernel(x, edge_index, W1, b1, W2, b2):
    global LAST_RESULTS
    meta, in_maps = preprocess(x, edge_index, W1, b1, W2, b2)
    key = (tuple(meta["cap"].reshape(-1)), meta["has_b1"], meta["has_b2"])
    if key not in _CACHE:
        _CACHE[key] = build(meta)
    nc = _CACHE[key]
    trace = bool(int(os.environ.get("GCN_TRACE", "0")))
    if trace:
        _enable_trace_shim()
    res = run_bass_kernel_spmd(
        nc, in_maps, core_ids=list(range(NCORES)), trace=trace
    )
    LAST_RESULTS = res
    return np.concatenate([res.results[c]["out"] for c in range(NCORES)], axis=0)
